# revision 6
# baseline (speedup 1.0000x reference)
# Trainium2 Bass kernel for nn_LogitsNew (dense_mlp).
#
#   u = gelu(x @ W_proj + b_proj)                       [B, D]
#   logits = (u @ W_u)[:, None, :] + ee @ W_e           [B, N, C]
#
# Sharding: data-parallel over batch B across 8 cores (4 batches/core).
# Per core:
#   - utterance path: xT via PE transpose, z = x@W_proj (+b via K=1 ones
#     matmul into the same PSUM group), u = Gelu(z) on ACT, y = u@W_u.
#   - main path: for each 128-row tile of ee, PE-transpose the 8 [128,128]
#     d-chunks, then accumulate eeT.T @ W_e into PSUM over the 8 k-tiles,
#     finally add the broadcast y row with one K=1 ones-matmul (stop=True),
#     copy PSUM->SBUF on DVE and DMA out.
# Matmuls run as float32r (full-rate fp32 path on the PE for free dim >= 256).

import sys

if "/opt/trn_rl_repo" not in sys.path:
    sys.path.insert(0, "/opt/trn_rl_repo")

import numpy as np

import concourse.bass as bass
import concourse.mybir as mybir
import concourse.tile as tile
from concourse import bacc
from concourse.bass_utils import run_bass_kernel_spmd
from concourse.masks import make_identity

P = 128
B, N, D, C = 32, 256, 1024, 1024
NCORES = 8
BPC = B // NCORES          # batches per core
KT = D // P                # 8 k-tiles over the contraction dim
FD = 512                   # matmul moving free dim (one PSUM bank of fp32)
NT = N // P                # 2 n-tiles per batch
MT = BPC * NT              # 8 m-tiles per core

F32 = mybir.dt.float32
F32R = mybir.dt.float32r
GELU = mybir.ActivationFunctionType.Gelu

_CACHE = {}


def _build():
    if "nc" in _CACHE:
        return _CACHE["nc"]

    nc = bacc.Bacc("TRN2", target_bir_lowering=False, debug=False, num_devices=NCORES)

    x = nc.dram_tensor("encoded_utterance", [BPC, D], F32R, kind="ExternalInput").ap()
    ee = nc.dram_tensor(
        "element_embeddings", [BPC, N, D], F32R, kind="ExternalInput"
    ).ap()
    w = nc.dram_tensor("weight_matrix", [2 * D, C], F32R, kind="ExternalInput").ap()
    wp = nc.dram_tensor("W_proj", [D, D], F32R, kind="ExternalInput").ap()
    bp = nc.dram_tensor("b_proj", [1, D], F32R, kind="ExternalInput").ap()
    out = nc.dram_tensor("logits", [BPC, N, C], F32, kind="ExternalOutput").ap()

    w3 = w.rearrange("(ko p) c -> p ko c", p=P)     # [128, 16, 1024]; ko 0..7 = W_u
    wp3 = wp.rearrange("(ko p) c -> p ko c", p=P)   # [128, 8, 1024]

    with tile.TileContext(nc) as tc:
        with (
            tc.tile_pool(name="const", bufs=1) as cpool,
            tc.tile_pool(name="weights", bufs=1) as wpool,
            tc.tile_pool(name="ee", bufs=2) as eepool,
            tc.tile_pool(name="eet", bufs=2) as eetpool,
            tc.tile_pool(name="outs", bufs=3) as outpool,
            tc.tile_pool(name="tp_ps", bufs=2, space="PSUM") as tp_ps,
            tc.tile_pool(name="mm_ps", bufs=4, space="PSUM") as mm_ps,
        ):
            # ---- constants / small inputs (SP ring) ----
            ident_f = cpool.tile([P, P], F32)
            make_identity(nc, ident_f)
            ident = cpool.tile([P, P], F32R)
            nc.scalar.copy(ident, ident_f)
            ones_f = cpool.tile([1, P], F32)
            nc.gpsimd.memset(ones_f, 1.0)
            ones = cpool.tile([1, P], F32R)
            nc.scalar.copy(ones, ones_f)
            x_sb = cpool.tile([BPC, D], F32R)
            nc.sync.dma_start(x_sb, x)
            b_sb = cpool.tile([1, D], F32R)
            nc.sync.dma_start(b_sb, bp)

            # ---- weights (ACT ring): W_e first (unblocks main loop), then
            # W_proj (unblocks z), then W_u (unblocks y). ----
            w_sb = wpool.tile([P, 2 * KT, C], F32R)
            wp_sb = wpool.tile([P, KT, C], F32R)
            nc.scalar.dma_start(w_sb[:, 8:12], w3[:, 8:12])
            nc.scalar.dma_start(w_sb[:, 12:16], w3[:, 12:16])
            nc.scalar.dma_start(wp_sb[:, 0:4], wp3[:, 0:4])
            nc.scalar.dma_start(wp_sb[:, 4:8], wp3[:, 4:8])
            nc.scalar.dma_start(w_sb[:, 0:4], w3[:, 0:4])
            nc.scalar.dma_start(w_sb[:, 4:8], w3[:, 4:8])

            # ---- utterance path ----
            xT = cpool.tile([P, KT, BPC], F32R)
            for k in range(KT):
                tp = tp_ps.tile([P, P], F32R, tag="tp")
                nc.tensor.transpose(
                    tp[:, :BPC], x_sb[:BPC, k * P : (k + 1) * P], ident[:BPC, :BPC]
                )
                nc.scalar.copy(xT[:, k, :], tp[:, :BPC])

            u_sb = cpool.tile([BPC, C], F32R)
            for h in range(2):
                cs = slice(h * FD, (h + 1) * FD)
                zp = mm_ps.tile([P, FD], F32, tag="mm")
                for k in range(KT):
                    nc.tensor.matmul(
                        zp[:BPC],
                        xT[:, k, :],
                        wp_sb[:, k, cs],
                        start=(k == 0),
                        stop=False,
                    )
                nc.tensor.matmul(
                    zp[:BPC],
                    ones[:1, :BPC],
                    b_sb[:1, cs],
                    start=False,
                    stop=True,
                )
                nc.scalar.activation(u_sb[:, cs], zp[:BPC], GELU)

            uT = cpool.tile([P, KT, BPC], F32R)
            for k in range(KT):
                tp = tp_ps.tile([P, P], F32R, tag="tp")
                nc.tensor.transpose(
                    tp[:, :BPC], u_sb[:BPC, k * P : (k + 1) * P], ident[:BPC, :BPC]
                )
                nc.scalar.copy(uT[:, k, :], tp[:, :BPC])

            y_sb = cpool.tile([BPC, C], F32R)
            for h in range(2):
                cs = slice(h * FD, (h + 1) * FD)
                yp = mm_ps.tile([P, FD], F32, tag="mm")
                for k in range(KT):
                    nc.tensor.matmul(
                        yp[:BPC],
                        uT[:, k, :],
                        w_sb[:, k, cs],
                        start=(k == 0),
                        stop=(k == KT - 1),
                    )
                nc.vector.tensor_copy(y_sb[:, cs], yp[:BPC])

            # y as a single row [1, BPC, C] so the broadcast matmul's rhs has
            # base partition 0.
            y_row = cpool.tile([1, BPC, C], F32R)
            nc.sync.dma_start(y_row, y_sb)

            # ---- main path ----
            for mt in range(MT):
                b, nh = divmod(mt, NT)
                ns = slice(nh * P, (nh + 1) * P)
                ee_t = eepool.tile([P, D], F32R, tag="ee")
                nc.sync.dma_start(ee_t, ee[b, ns, :])
                eet = eetpool.tile([P, KT, P], F32R, tag="eet")
                for k in range(KT):
                    tp = tp_ps.tile([P, P], F32R, tag="tp")
                    nc.tensor.transpose(tp, ee_t[:, k * P : (k + 1) * P], ident)
                    nc.scalar.copy(eet[:, k, :], tp)
                for ch in range(2):
                    cs = slice(ch * FD, (ch + 1) * FD)
                    mp = mm_ps.tile([P, FD], F32, tag="mm")
                    for k in range(KT):
                        nc.tensor.matmul(
                            mp,
                            eet[:, k, :],
                            w_sb[:, KT + k, cs],
                            start=(k == 0),
                            stop=False,
                        )
                    nc.tensor.matmul(
                        mp,
                        ones[:1, :P],
                        y_row[:1, b, cs],
                        start=False,
                        stop=True,
                    )
                    o = outpool.tile([P, FD], F32, tag="o")
                    nc.vector.tensor_copy(o, mp)
                    nc.sync.dma_start(out[b, ns, cs], o)

    nc.compile()
    _CACHE["nc"] = nc
    return nc


def run(inputs, trace=False, **kwargs):
    nc = _build()
    x = np.ascontiguousarray(np.asarray(inputs["encoded_utterance"], np.float32))
    ee = np.ascontiguousarray(np.asarray(inputs["element_embeddings"], np.float32))
    w = np.ascontiguousarray(np.asarray(inputs["weight_matrix"], np.float32))
    wp = np.ascontiguousarray(np.asarray(inputs["W_proj"], np.float32))
    bp = np.ascontiguousarray(
        np.asarray(inputs["b_proj"], np.float32).reshape(1, D)
    )

    in_maps = []
    for i in range(NCORES):
        bs = slice(i * BPC, (i + 1) * BPC)
        in_maps.append(
            {
                "encoded_utterance": x[bs],
                "element_embeddings": ee[bs],
                "weight_matrix": w,
                "W_proj": wp,
                "b_proj": bp,
            }
        )

    res = run_bass_kernel_spmd(
        nc, in_maps, core_ids=list(range(NCORES)), trace=trace, **kwargs
    )
    full = np.concatenate([r["logits"] for r in res.results], axis=0)
    return full, res


def kernel(**inputs) -> np.ndarray:
    return run(inputs, trace=False)[0]
